# revision 28
# baseline (speedup 1.0000x reference)
"""Trainium2 Bass kernel for a dense transformer block (nn_Block_3564822855835).

Reference computation (fp32):
    x  = x + attention(rmsnorm(x, g1), Wq, Wk, Wv, Wo)   # causal MHA, 16 heads
    out = x + gelu(rmsnorm(x, g2) @ W1 + b1) @ W2 + b2   # exact-erf gelu

Shapes: x [2, 2048, 1024], 16 heads x 64, d_ff 4096.

Distribution (8 NeuronCores, one SPMD program, DeepSpeed-Ulysses style):
  - Token-sharded outside attention: core c owns batch c//4, token block
    c%4 (512 tokens): rmsnorm + q/k/v projections + o-proj + MLP all run
    on the core's own 512 tokens with full (replicated) weights.
  - Head-sharded inside attention: an 8-core AllToAll redistributes
    (q^T, k^T, v) from token-shards to head-shards; each core then runs
    causal attention for its 2 heads over the full 4096 tokens, and a
    second AllToAll routes y^T back to token shards.
  - Activations are channel-major ("x^T": [channel, token]) on-chip so all
    matmul contractions land on the partition axis with zero on-chip
    transposes.  Partition-dim reductions (rmsnorm sum, softmax sum) use
    a ones-column fused into the matmuls; scale rows are partition-
    broadcast on GPSIMD.
  - Diagonal causal chunks only compute the live query suffix: the score
    matmuls and the exp stream skip the fully-masked query prefix (the
    mask multiply still zeroes it for the AV accumulation).
  - Optionally (FP8_QKV) the q/k/v projections run in fp8 e4m3 with
    DoubleRow perf mode (2 contraction rows per PE pass, measured 2.2x
    matmul throughput on HW); weights are host-scaled by WS=128 and the
    descale rides the exp scale (q,k) and the softmax-normalizer row (v).
    The MLP and o-proj stay bf16: e4m3's ~4% per-matmul noise there takes
    the output past the accuracy budget.
  - Matmul operands are bf16/fp8 (fp32 PSUM accumulation); the residual
    stream and all softmax/norm statistics stay fp32.
"""

import numpy as np
import ml_dtypes

import concourse.bass as bass
import concourse.mybir as mybir
import concourse.tile as tile
from concourse import bacc

F32 = mybir.dt.float32
F32R = mybir.dt.float32r
BF16 = mybir.dt.bfloat16
F8 = mybir.dt.float8e4
AF = mybir.ActivationFunctionType
DRmode = mybir.MatmulPerfMode.DoubleRow

B, T, D = 2, 2048, 1024
H, DH = 16, 64
FF = 4096
EPS = 1e-6
P = 128
N_CORES = 8
NT = 512            # tokens per core
CC = D // P         # 8 channel chunks
NCH = T // P        # 16 k-chunks per batch
FT = FF // P        # 32 ff tiles
PNT = P * NT        # elements in one [128, 512] plane

WS = 128.0          # host-side fp8 weight scale (fp8 qkv mode)
FP8_QKV = True      # final config: q/k/v projections in fp8-DoubleRow

ALL8 = [[0, 1, 2, 3, 4, 5, 6, 7]]


ABLATE = set()


def _build_nc(with_collective=True, repeat=1, fp8_qkv=FP8_QKV):
    nc = bacc.Bacc("TRN2", target_bir_lowering=False, debug=False,
                   num_devices=N_CORES)

    def inp(name, shape, dt=F32):
        return nc.dram_tensor(name, shape, dt, kind="ExternalInput").ap()

    WDT = F8 if fp8_qkv else BF16
    xt = inp("xt", [D, NT])                 # x slice, channel-major
    # weights are host-pre-tiled so every DMA reads fully contiguous blocks
    wq = inp("wq", [CC, P, CC, P], WDT)     # [dt, p, cc, dd], g1-folded (x WS)
    wk = inp("wk", [CC, P, CC, P], WDT)     # [dt, p, cc, dd], g1-folded (x WS)
    wv = inp("wv", [D, D], WDT)             # g1-folded (x WS)
    wo = inp("wo", [P, CC, D], BF16)        # host-pretiled [p, cc, c]
    w1 = inp("w1", [FT, P, CC, P], BF16)    # [ff, p, cc, dd], g2-folded
    w2 = inp("w2", [CC, 2, P, FT // 2, P], BF16)  # [ct, half, p, fo, c]
    b1 = inp("b1", [P, FT])
    b2 = inp("b2", [P, CC])
    dmask = inp("dmask", [4, P, NT], BF16)  # diagonal causal masks (global)
    onesc = inp("onesc", [P, 1])
    outT = nc.dram_tensor("outT", [D, NT], F32, kind="ExternalOutput").ap()

    with tile.TileContext(nc) as tc:
        with tc.tile_pool(name="const", bufs=1) as constp, \
             tc.tile_pool(name="actsf", bufs=1) as actsf, \
             tc.tile_pool(name="actsr", bufs=3) as actsr, \
             tc.tile_pool(name="hpool", bufs=1) as hpool, \
             tc.tile_pool(name="wstr", bufs=2) as wstr, \
             tc.tile_pool(name="wstr2", bufs=2) as wstr2, \
             tc.tile_pool(name="kvp", bufs=6) as kvp, \
             tc.tile_pool(name="expp", bufs=5) as expp, \
             tc.tile_pool(name="cpp", bufs=3) as cpp, \
             tc.tile_pool(name="smallp", bufs=2) as smallp, \
             tc.tile_pool(name="rowp", bufs=2) as rowp, \
             tc.tile_pool(name="bcp", bufs=1) as bcp, \
             tc.tile_pool(name="psb", bufs=2, space="PSUM") as psb, \
             tc.tile_pool(name="pss", bufs=2, space="PSUM") as pss, \
             tc.tile_pool(name="psy", bufs=2, space="PSUM") as psy, \
             tc.tile_pool(name="dram", bufs=1, space="DRAM") as dram:

            # ---------------- constants ----------------
            onesc_sb = constp.tile([P, 1], F32R)
            nc.sync.dma_start(onesc_sb[:], onesc[:].bitcast(F32R))
            b1_sb = constp.tile([P, FT], F32)
            nc.sync.dma_start(b1_sb[:], b1[:])
            b2_sb = constp.tile([P, CC], F32)
            nc.sync.dma_start(b2_sb[:], b2[:])
            eps_sb = constp.tile([1, 1], F32)
            nc.vector.memset(eps_sb[:], EPS)
            onesr_sb = constp.tile([1, P], F32)
            nc.sync.dma_start(onesr_sb[:], onesc.rearrange("p one -> one p"))

            # warm all 5 es-pool buffers: diagonal score chunks only write the
            # live query suffix, and the masked prefix must multiply against
            # finite stale data (0 x NaN would poison the AV accumulation)
            for _ in range(5):
                es2 = expp.tile([P, 2, NT], BF16, tag="es")
                nc.vector.memset(es2[:], 0.0)

            for _rep in range(repeat):
                xT = actsf.tile([P, CC, NT], F32, tag="bigf")
                nc.sync.dma_start(xT[:], xt.rearrange("(cc p) t -> p cc t", p=P))

                # ---------------- rmsnorm (channel-major) ----------------
                def rmsnorm(src_sb, dt_out):
                    """src [P, CC, NT] f32 -> normalized [P, CC, NT] dt_out."""
                    prow = psb.tile([1, NT], F32, tag="pbig")
                    for cc in range(CC):
                        sq = smallp.tile([P, NT], F32R, tag="sq")
                        nc.vector.tensor_mul(sq[:], src_sb[:, cc, :], src_sb[:, cc, :])
                        nc.tensor.matmul(prow[:], onesc_sb[:], sq[:],
                                         start=(cc == 0), stop=(cc == CC - 1))
                    srow = rowp.tile([1, NT], F32, tag="row")
                    nc.scalar.activation(srow[:], prow[:], AF.Sqrt,
                                         bias=eps_sb[:], scale=1.0 / D)
                    rrow = rowp.tile([1, NT], F32, tag="row")
                    nc.vector.reciprocal(rrow[:], srow[:])
                    # partition-broadcast via K=1 matmul + ACT copy: avoids a
                    # GPSIMD round-trip on the critical path into the matmuls
                    bps = pss.tile([P, 2, NT], F32, tag="ps")
                    nc.tensor.matmul(bps[:, 0, :], onesr_sb[:], rrow[:],
                                     start=True, stop=True)
                    bc = bcp.tile([P, NT], F32, tag="bc")
                    nc.scalar.activation(bc[:], bps[:, 0, :], AF.Copy)
                    dst = actsr.tile([P, CC, NT], dt_out, tag="bigr")
                    for cc in range(CC):
                        nc.vector.tensor_mul(dst[:, cc, :], src_sb[:, cc, :], bc[:])
                    return dst

                xn = rmsnorm(xT, WDT)

                # ------------- q/k/v projections -> AllToAll bounce -------------
                # shard j (head pair j), row-major per partition p:
                #   [q^T 512 | k^T 512 | v 4x(2x65)]  (1544 bf16 per p-row)
                # one contiguous DMA per (batch, rank) on the attention side.
                # slot 64 of each v 65-group becomes 1.0 via on-chip memset.
                VW = 2 * 65          # 130 bf16 per (p, c)
                QKW = 2 * NT                   # 1024 bf16 per p-row (q|k)
                VVW = 4 * VW                   # 520 bf16 per p-row (v)
                qk_in = dram.tile([N_CORES, P * QKW], BF16)
                qk_out = dram.tile([N_CORES, P * QKW], BF16)
                v_in = dram.tile([N_CORES, P * VVW], BF16)
                v_out = dram.tile([N_CORES, P * VVW], BF16)

                def wmm(pb, wt_sl, xn_sl_pairs, j, njj):
                    """one contraction step: DR pair in fp8 mode, single bf16."""
                    if fp8_qkv:
                        nc.tensor.matmul(pb, wt_sl, xn_sl_pairs,
                                         start=(j == 0), stop=(j == njj - 1),
                                         perf_mode=DRmode)
                    else:
                        nc.tensor.matmul(pb, wt_sl, xn_sl_pairs,
                                         start=(j == 0), stop=(j == njj - 1))

                def proj_qk(w, region):
                    # whole weight tensor in one DMA (8 KB per partition)
                    wt = wstr.tile([P, CC, CC, P], WDT, tag="w81d", bufs=1)
                    nc.sync.dma_start(
                        wt[:], w.rearrange("f p c d -> p f (c d)"))
                    qcol = cpp.tile([P, CC, NT], BF16, tag="qcol", bufs=1)
                    for dt in range(CC):
                        pb = psb.tile([P, NT], F32, tag="pbig")
                        if fp8_qkv:
                            for j in range(CC // 2):
                                wmm(pb[:], wt[:, dt, 2 * j:2 * j + 2, :],
                                    xn[:, 2 * j:2 * j + 2, :], j, CC // 2)
                        else:
                            for cc in range(CC):
                                wmm(pb[:], wt[:, dt, cc, :],
                                    xn[:, cc, :], cc, CC)
                        nc.vector.tensor_copy(qcol[:, dt, :], pb[:])
                    # all 8 head-pair shards in one DMA
                    nc.sync.dma_start(
                        qk_in[:, :].rearrange("r (p z) -> p r z", z=QKW)
                        [:, :, region * NT:(region + 1) * NT], qcol[:])

                if "qkv" not in ABLATE:
                    proj_qk(wq, 0)
                    proj_qk(wk, 1)
                if with_collective:
                    nc.gpsimd.collective_compute(
                        "AllToAll", mybir.AluOpType.bypass, replica_groups=ALL8,
                        ins=[qk_in[:].opt()], outs=[qk_out[:].opt()])
                else:
                    nc.sync.dma_start(qk_out[:], qk_in[:])

                # v token-major: v[t, d] = sum_c xn[c, t] wv[c, d]
                for dt in range(2 if "qkv" not in ABLATE else 0):
                    wt = wstr2.tile([P, CC, NT], WDT, tag="w82")
                    nc.sync.dma_start(
                        wt[:], wv[:, dt * NT:(dt + 1) * NT]
                        .rearrange("(cc p) d -> p cc d", p=P))
                    for tt in range(4):
                        pb = psb.tile([P, NT], F32, tag="pbig")
                        if fp8_qkv:
                            for j in range(CC // 2):
                                wmm(pb[:],
                                    xn[:, 2 * j:2 * j + 2, tt * P:(tt + 1) * P],
                                    wt[:, 2 * j:2 * j + 2, :], j, CC // 2)
                        else:
                            for cc in range(CC):
                                wmm(pb[:], xn[:, cc, tt * P:(tt + 1) * P],
                                    wt[:, cc, :], cc, CC)
                        cp = cpp.tile([P, NT], BF16, tag="cpb")
                        nc.vector.tensor_copy(cp[:], pb[:])
                        for hh in range(2):
                            vdst = (v_in[4 * dt:4 * dt + 4]
                                    .rearrange("u (p z) -> p u z", z=VVW)
                                    [:, :, tt * VW:(tt + 1) * VW]
                                    .rearrange("p u (h e) -> p u h e", e=65)
                                    [:, :, hh, 0:DH])
                            nc.sync.dma_start(
                                vdst, cp[:].rearrange("p (u h d) -> p h u d",
                                                      h=2, d=DH)[:, hh])

                if with_collective:
                    nc.gpsimd.collective_compute(
                        "AllToAll", mybir.AluOpType.bypass, replica_groups=ALL8,
                        ins=[v_in[:].opt()], outs=[v_out[:].opt()])
                else:
                    nc.sync.dma_start(v_out[:], v_in[:])

                # loads that are only needed later: their DMA overlaps the
                # collective / the attention window instead of the qkv window
                mask_sb = constp.tile([P, 4, NT], BF16)
                nc.sync.dma_start(mask_sb[:], dmask.rearrange("g p q -> p g q"))
                wo_sb = constp.tile([P, CC, D], BF16)
                nc.sync.dma_start(wo_sb[:], wo[:])

                # ------------- attention (my 2 heads, all tokens) -------------
                a2a2_in = dram.tile([N_CORES, PNT], BF16)
                a2a2_out = dram.tile([N_CORES, PNT], BF16)
                yTm = actsr.tile([P, CC, NT], BF16, tag="bigr")
                exp_scale = 0.125 / (WS * WS) if fp8_qkv else 0.125
                # hoist both batches' loads ahead of any attention compute:
                # the in-order SP queue would otherwise park batch 1's loads
                # behind batch 0's y-write semaphore wait
                kq_ts, vv_ts = [], []
                for b in range(B):
                    kq_t = kvp.tile([P, 4, QKW], BF16, tag="kq", bufs=2)
                    nc.sync.dma_start(
                        kq_t[:], qk_out[4 * b:4 * b + 4]
                        .rearrange("r (p z) -> p r z", z=QKW))
                    vv_t = kvp.tile([P, 4, VVW], BF16, tag="vv", bufs=2)
                    nc.sync.dma_start(
                        vv_t[:], v_out[4 * b:4 * b + 4]
                        .rearrange("r (p z) -> p r z", z=VVW))
                    nc.vector.memset(
                        vv_t[:].rearrange("p r (c h e) -> p (r c h) e",
                                          h=2, e=65)
                        [:, :, DH:DH + 1], 1.0)
                    kq_ts.append(kq_t)
                    vv_ts.append(vv_t)
                for b in range(B):
                    kq_t = kq_ts[b]
                    vv_t = vv_ts[b]

                    for qb in range(4):
                        if "attn" in ABLATE:
                            continue
                        py0 = psy.tile([DH + 1, NT], F32, tag="py")
                        py1 = psy.tile([DH + 1, NT], F32, tag="py")
                        pys = (py0, py1)
                        nch = 4 * qb + 4
                        for g in range(nch):
                            # both heads' scores side by side in one 2-bank tile:
                            # the K=64 matmuls run in disjoint PE row groups and
                            # one ACT exp covers both.  Diagonal chunks only
                            # compute the live query suffix [qo:].
                            qo = P * (g - 4 * qb) if g >= 4 * qb else 0
                            rr = g // 4
                            ko = NT + (g % 4) * P
                            ps2 = pss.tile([P, 2, NT], F32, tag="ps")
                            nc.tensor.matmul(ps2[:, 0, qo:],
                                             kq_t[0:64, rr, ko:ko + P],
                                             kq_t[0:64, qb, qo:NT],
                                             start=True, stop=True)
                            nc.tensor.matmul(ps2[:, 1, qo:],
                                             kq_t[64:128, rr, ko:ko + P],
                                             kq_t[64:128, qb, qo:NT],
                                             start=True, stop=True)
                            es2 = expp.tile([P, 2, NT], BF16, tag="es")
                            nc.scalar.activation(es2[:, :, qo:], ps2[:, :, qo:],
                                                 AF.Exp, scale=exp_scale)
                            if g >= 4 * qb:
                                d = g - 4 * qb
                                nc.vector.tensor_mul(
                                    es2[:], es2[:],
                                    mask_sb[:, d:d + 1, :].to_broadcast([P, 2, NT]))
                            vo = (g % 4) * VW
                            nc.tensor.matmul(py0[:], vv_t[:, rr, vo:vo + 65],
                                             es2[:, 0, :],
                                             start=(g == 0), stop=(g == nch - 1))
                            nc.tensor.matmul(py1[:], vv_t[:, rr, vo + 65:vo + VW],
                                             es2[:, 1, :],
                                             start=(g == 0), stop=(g == nch - 1))
                        for hh in range(2):
                            lo = 64 * hh
                            pc = smallp.tile([DH + 1, NT], F32, tag="pyc")
                            nc.vector.tensor_copy(pc[:], pys[hh][:])
                            rrow = rowp.tile([1, NT], F32, tag="row")
                            nc.vector.reciprocal(rrow[:], pc[DH:DH + 1, :])
                            if fp8_qkv:
                                # v carries the WS weight scale; fold 1/WS into
                                # the per-token normalizer row
                                nc.vector.tensor_scalar_mul(rrow[:], rrow[:],
                                                            1.0 / WS)
                            bcy = smallp.tile([64, NT], F32, tag="abc")
                            nc.gpsimd.partition_broadcast(bcy[:], rrow[:])
                            nc.vector.tensor_mul(
                                yTm[lo:lo + 64, 4 * b + qb, :],
                                pc[0:DH, :], bcy[:])
                    if "attn" not in ABLATE:
                        nc.sync.dma_start(
                            a2a2_in[4 * b:4 * b + 4].rearrange(
                                "r (p t) -> p r t", t=NT),
                            yTm[:, 4 * b:4 * b + 4, :])

                # ------------- AllToAll y back to token shards -------------
                if "attn" in ABLATE:
                    nc.vector.memset(yTm[:], 0.0)
                    nc.sync.dma_start(
                        a2a2_in[:, :].rearrange("r (p t) -> p r t", t=NT), yTm[:])
                if with_collective:
                    nc.gpsimd.collective_compute(
                        "AllToAll", mybir.AluOpType.bypass, replica_groups=ALL8,
                        ins=[a2a2_in[:].opt()], outs=[a2a2_out[:].opt()])
                else:
                    nc.sync.dma_start(a2a2_out[:], a2a2_in[:])
                yTf = actsr.tile([P, CC, NT], BF16, tag="bigr")
                nc.sync.dma_start(
                    yTf[:], a2a2_out[:, :].rearrange("r (p t) -> p r t", t=NT))

                # ------------- o-proj + residual (in place into xT) -------------
                for ct in range(CC):
                    pb = psb.tile([P, NT], F32, tag="pbig")
                    for cc in range(CC):
                        nc.tensor.matmul(pb[:], wo_sb[:, cc, ct * P:(ct + 1) * P],
                                         yTf[:, cc, :],
                                         start=(cc == 0), stop=(cc == CC - 1))
                    nc.vector.tensor_add(xT[:, ct, :], pb[:], xT[:, ct, :])

                # ---------------- rmsnorm2 + fc1 + gelu ----------------
                xn2 = rmsnorm(xT, BF16)
                hT = hpool.tile([P, FT, NT], BF16)
                if "mlp" in ABLATE:
                    nc.vector.memset(hT[:, 0, :], 0.0)
                for fq in range(FT // 4 if "mlp" not in ABLATE else 0):
                    wt = wstr.tile([P, 4, CC, P], BF16, tag="w81q")
                    nc.sync.dma_start(
                        wt[:], w1[4 * fq:4 * fq + 4]
                        .rearrange("f p c d -> p f (c d)"))
                    for f4 in range(4):
                        ff = 4 * fq + f4
                        pb = psb.tile([P, NT], F32, tag="pbig")
                        for cc in range(CC):
                            nc.tensor.matmul(pb[:], wt[:, f4, cc, :], xn2[:, cc, :],
                                             start=(cc == 0), stop=(cc == CC - 1))
                        nc.scalar.activation(hT[:, ff, :], pb[:], AF.Gelu,
                                             bias=b1_sb[:, ff:ff + 1])

                # ---------------- fc2 + bias + residual ----------------
                # written in place into xT (dead after its last fc2 read),
                # then one merged store
                for ct in range(CC if "mlp" not in ABLATE else 0):
                    pb = psb.tile([P, NT], F32, tag="pbig")
                    wt = wstr2.tile([P, 2, FT // 2, P], BF16, tag="w16")
                    nc.sync.dma_start(
                        wt[:], w2[ct].rearrange("h p f c -> p h (f c)"))
                    for half in range(2):
                        for fo in range(FT // 2):
                            ffc = half * (FT // 2) + fo
                            nc.tensor.matmul(pb[:], wt[:, half, fo, :],
                                             hT[:, ffc, :],
                                             start=(ffc == 0), stop=(ffc == FT - 1))
                    nc.vector.tensor_add(xT[:, ct, :], pb[:], xT[:, ct, :])
                    nc.vector.tensor_scalar_add(xT[:, ct, :], xT[:, ct, :],
                                                b2_sb[:, ct:ct + 1])
                nc.sync.dma_start(
                    outT.rearrange("(cc p) t -> p cc t", p=P), xT[:])

    nc.compile()
    return nc


_CACHE = {}


def _get_compiled(with_collective=True, repeat=1, fp8_qkv=FP8_QKV):
    """Build the Bass program and a jitted 8-core PJRT executable once."""
    key = ("fn", with_collective, repeat, fp8_qkv)
    if key in _CACHE:
        return _CACHE[key]

    import jax
    from jax.sharding import Mesh, PartitionSpec
    from jax.experimental.shard_map import shard_map
    from concourse.bass2jax import (_bass_exec_p, install_neuronx_cc_hook,
                                    partition_id_tensor)

    nc = _build_nc(with_collective, repeat, fp8_qkv)
    install_neuronx_cc_hook()

    partition_name = (nc.partition_id_tensor.name
                      if nc.partition_id_tensor else None)
    in_names, out_names, out_avals = [], [], []
    for alloc in nc.m.functions[0].allocations:
        if not isinstance(alloc, mybir.MemoryLocationSet):
            continue
        name = alloc.memorylocations[0].name
        if alloc.kind == "ExternalInput":
            if name != partition_name:
                in_names.append(name)
        elif alloc.kind == "ExternalOutput":
            out_names.append(name)
            out_avals.append(jax.core.ShapedArray(
                tuple(alloc.tensor_shape), mybir.dt.np(alloc.dtype)))
    n_params = len(in_names)
    all_names = list(in_names) + list(out_names)
    if partition_name is not None:
        all_names.append(partition_name)

    def _body(*args):
        operands = list(args)
        if partition_name is not None:
            operands.append(partition_id_tensor())
        outs = _bass_exec_p.bind(
            *operands,
            out_avals=tuple(out_avals),
            in_names=tuple(all_names),
            out_names=tuple(out_names),
            lowering_input_output_aliases=(),
            sim_require_finite=True,
            sim_require_nnan=True,
            nc=nc,
        )
        return tuple(outs)

    devices = jax.devices()[:N_CORES]
    mesh = Mesh(np.asarray(devices), ("core",))
    sharded = jax.jit(shard_map(
        _body, mesh=mesh,
        in_specs=(PartitionSpec("core"),) * (n_params + len(out_names)),
        out_specs=(PartitionSpec("core"),) * len(out_names),
        check_rep=False))

    _CACHE[key] = (sharded, in_names, out_names, out_avals)
    return _CACHE[key]


def _host_inputs(x, Wq, Wk, Wv, Wo, W1, b1, W2, b2, g1, g2, fp8_qkv=FP8_QKV):
    """Per-core input dicts (all keys identically shaped across cores)."""
    bf = ml_dtypes.bfloat16
    f8 = ml_dtypes.float8_e4m3

    def tile8(w, dt):  # [D, out] -> [out_tiles, p, cc, 128] contiguous
        nt_ = w.shape[1] // P
        return np.ascontiguousarray(
            w.reshape(CC, P, nt_, P).transpose(2, 1, 0, 3)).astype(dt)

    if fp8_qkv:
        wq = tile8(WS * g1[:, None] * Wq, f8)
        wk = tile8(WS * g1[:, None] * Wk, f8)
        wv = (WS * g1[:, None] * Wv).astype(f8)
    else:
        wq = tile8(g1[:, None] * Wq, bf)
        wk = tile8(g1[:, None] * Wk, bf)
        wv = (g1[:, None] * Wv).astype(bf)
    w1 = tile8(g2[:, None] * W1, bf)
    w2 = np.ascontiguousarray(
        W2.reshape(2, FT // 2, P, CC, P).transpose(3, 0, 2, 1, 4)).astype(bf)
    wo = np.ascontiguousarray(
        Wo.reshape(CC, P, D).transpose(1, 0, 2)).astype(bf)
    b1 = np.ascontiguousarray(b1.reshape(FT, P).T).astype(np.float32)
    b2 = np.ascontiguousarray(b2.reshape(CC, P).T).astype(np.float32)
    onesc = np.ones((P, 1), np.float32)
    k_idx = np.arange(P)[:, None]
    q_idx = np.arange(NT)[None, :]
    dmask = np.stack([(P * d + k_idx <= q_idx) for d in range(4)]).astype(bf)

    per_core = []
    for c in range(N_CORES):
        b, s = divmod(c, 4)
        xt = np.ascontiguousarray(
            x[b, s * NT:(s + 1) * NT, :].T).astype(np.float32)
        per_core.append(dict(
            xt=xt, wq=wq, wk=wk, wv=wv, wo=wo,
            w1=w1, w2=w2, b1=b1, b2=b2,
            dmask=dmask, onesc=onesc))
    return per_core


def _concat_inputs(per_core, in_names, out_avals):
    concat = [np.concatenate([np.asarray(per_core[c][n])
                              for c in range(N_CORES)], axis=0)
              for n in in_names]
    concat += [np.zeros((N_CORES * a.shape[0], *a.shape[1:]), a.dtype)
               for a in out_avals]
    return concat


def _run(per_core):
    sharded, in_names, out_names, out_avals = _get_compiled()
    outs = sharded(*_concat_inputs(per_core, in_names, out_avals))
    res = np.asarray(outs[out_names.index("outT")])
    return res.reshape(N_CORES, D, NT)


def kernel(**inputs):
    np_in = {k: np.asarray(v) for k, v in inputs.items()}
    per_core = _host_inputs(**np_in)
    res = _run(per_core)
    out = np.empty((B, T, D), np.float32)
    for c in range(N_CORES):
        b, s = divmod(c, 4)
        out[b, s * NT:(s + 1) * NT, :] = res[c].T
    return out


def _bench_handles(inputs, with_collective=True, repeat=1, fp8_qkv=FP8_QKV):
    """Compiled runner + device-resident args, for benchmarking."""
    per_core = _host_inputs(
        **{k: np.asarray(v) for k, v in inputs.items()}, fp8_qkv=fp8_qkv)
    sharded, in_names, out_names, out_avals = _get_compiled(
        with_collective, repeat, fp8_qkv)
    import jax
    dev_args = [jax.device_put(a)
                for a in _concat_inputs(per_core, in_names, out_avals)]
    return sharded, dev_args


# revision 29
# speedup vs baseline: 1.1517x; 1.1517x over previous
"""Trainium2 Bass kernel for a dense transformer block (nn_Block_3564822855835).

Reference computation (fp32):
    x  = x + attention(rmsnorm(x, g1), Wq, Wk, Wv, Wo)   # causal MHA, 16 heads
    out = x + gelu(rmsnorm(x, g2) @ W1 + b1) @ W2 + b2   # exact-erf gelu

Shapes: x [2, 2048, 1024], 16 heads x 64, d_ff 4096.

Distribution (8 NeuronCores, one SPMD program, DeepSpeed-Ulysses style):
  - Token-sharded outside attention: core c owns batch c//4, token block
    c%4 (512 tokens): rmsnorm + q/k/v projections + o-proj + MLP all run
    on the core's own 512 tokens with full (replicated) weights.
  - Head-sharded inside attention: an 8-core AllToAll redistributes
    (q^T, k^T, v) from token-shards to head-shards; each core then runs
    causal attention for its 2 heads over the full 4096 tokens, and a
    second AllToAll routes y^T back to token shards.
  - Activations are channel-major ("x^T": [channel, token]) on-chip so all
    matmul contractions land on the partition axis with zero on-chip
    transposes.  Partition-dim reductions (rmsnorm sum, softmax sum) use
    a ones-column fused into the matmuls; scale rows are partition-
    broadcast on GPSIMD.
  - Diagonal causal chunks only compute the live query suffix: the score
    matmuls and the exp stream skip the fully-masked query prefix (the
    mask multiply still zeroes it for the AV accumulation).
  - Optionally (FP8_QKV) the q/k/v projections run in fp8 e4m3 with
    DoubleRow perf mode (2 contraction rows per PE pass, measured 2.2x
    matmul throughput on HW); weights are host-scaled by WS=128 and the
    descale rides the exp scale (q,k) and the softmax-normalizer row (v).
    The MLP and o-proj stay bf16: e4m3's ~4% per-matmul noise there takes
    the output past the accuracy budget.
  - Matmul operands are bf16/fp8 (fp32 PSUM accumulation); the residual
    stream and all softmax/norm statistics stay fp32.
"""

import numpy as np
import ml_dtypes

import concourse.bass as bass
import concourse.mybir as mybir
import concourse.tile as tile
from concourse import bacc

F32 = mybir.dt.float32
F32R = mybir.dt.float32r
BF16 = mybir.dt.bfloat16
F8 = mybir.dt.float8e4
AF = mybir.ActivationFunctionType
DRmode = mybir.MatmulPerfMode.DoubleRow

B, T, D = 2, 2048, 1024
H, DH = 16, 64
FF = 4096
EPS = 1e-6
P = 128
N_CORES = 8
NT = 512            # tokens per core
CC = D // P         # 8 channel chunks
NCH = T // P        # 16 k-chunks per batch
FT = FF // P        # 32 ff tiles
PNT = P * NT        # elements in one [128, 512] plane

WS = 128.0          # host-side fp8 weight scale (fp8 qkv mode)
FP8_QKV = True      # final config: q/k/v projections in fp8-DoubleRow

ALL8 = [[0, 1, 2, 3, 4, 5, 6, 7]]


ABLATE = set()


def _build_nc(with_collective=True, repeat=1, fp8_qkv=FP8_QKV):
    nc = bacc.Bacc("TRN2", target_bir_lowering=False, debug=False,
                   num_devices=N_CORES)

    def inp(name, shape, dt=F32):
        return nc.dram_tensor(name, shape, dt, kind="ExternalInput").ap()

    WDT = F8 if fp8_qkv else BF16
    xt = inp("xt", [D, NT])                 # x slice, channel-major
    # weights are host-pre-tiled so every DMA reads fully contiguous blocks
    wq = inp("wq", [CC, P, CC, P], WDT)     # [dt, p, cc, dd], g1-folded (x WS)
    wk = inp("wk", [CC, P, CC, P], WDT)     # [dt, p, cc, dd], g1-folded (x WS)
    wv = inp("wv", [D, D], WDT)             # g1-folded (x WS)
    wo = inp("wo", [P, CC, D], BF16)        # host-pretiled [p, cc, c]
    w1 = inp("w1", [FT, P, CC, P], BF16)    # [ff, p, cc, dd], g2-folded
    w2 = inp("w2", [CC, 2, P, FT // 2, P], BF16)  # [ct, half, p, fo, c]
    b1 = inp("b1", [P, FT])
    b2 = inp("b2", [P, CC])
    dmask = inp("dmask", [4, P, NT], BF16)  # diagonal causal masks (global)
    onesc = inp("onesc", [P, 1])
    outT = nc.dram_tensor("outT", [D, NT], F32, kind="ExternalOutput").ap()

    with tile.TileContext(nc) as tc:
        with tc.tile_pool(name="const", bufs=1) as constp, \
             tc.tile_pool(name="actsf", bufs=1) as actsf, \
             tc.tile_pool(name="actsr", bufs=3) as actsr, \
             tc.tile_pool(name="hpool", bufs=1) as hpool, \
             tc.tile_pool(name="wstr", bufs=2) as wstr, \
             tc.tile_pool(name="wstr2", bufs=2) as wstr2, \
             tc.tile_pool(name="kvp", bufs=6) as kvp, \
             tc.tile_pool(name="expp", bufs=5) as expp, \
             tc.tile_pool(name="cpp", bufs=3) as cpp, \
             tc.tile_pool(name="smallp", bufs=2) as smallp, \
             tc.tile_pool(name="rowp", bufs=2) as rowp, \
             tc.tile_pool(name="bcp", bufs=1) as bcp, \
             tc.tile_pool(name="psb", bufs=2, space="PSUM") as psb, \
             tc.tile_pool(name="pss", bufs=2, space="PSUM") as pss, \
             tc.tile_pool(name="psy", bufs=2, space="PSUM") as psy, \
             tc.tile_pool(name="dram", bufs=1, space="DRAM") as dram:

            # ---------------- constants ----------------
            onesc_sb = constp.tile([P, 1], F32R)
            nc.sync.dma_start(onesc_sb[:], onesc[:].bitcast(F32R))
            b1_sb = constp.tile([P, FT], F32)
            nc.sync.dma_start(b1_sb[:], b1[:])
            b2_sb = constp.tile([P, CC], F32)
            nc.sync.dma_start(b2_sb[:], b2[:])
            eps_sb = constp.tile([1, 1], F32)
            nc.vector.memset(eps_sb[:], EPS)

            # warm all 5 es-pool buffers: diagonal score chunks only write the
            # live query suffix, and the masked prefix must multiply against
            # finite stale data (0 x NaN would poison the AV accumulation)
            for _ in range(5):
                es2 = expp.tile([P, 2, NT], BF16, tag="es")
                nc.vector.memset(es2[:], 0.0)

            for _rep in range(repeat):
                xT = actsf.tile([P, CC, NT], F32, tag="bigf")
                nc.sync.dma_start(xT[:], xt.rearrange("(cc p) t -> p cc t", p=P))

                # ---------------- rmsnorm (channel-major) ----------------
                def rmsnorm(src_sb, dt_out):
                    """src [P, CC, NT] f32 -> normalized [P, CC, NT] dt_out."""
                    prow = psb.tile([1, NT], F32, tag="pbig")
                    for cc in range(CC):
                        sq = smallp.tile([P, NT], F32R, tag="sq")
                        nc.vector.tensor_mul(sq[:], src_sb[:, cc, :], src_sb[:, cc, :])
                        nc.tensor.matmul(prow[:], onesc_sb[:], sq[:],
                                         start=(cc == 0), stop=(cc == CC - 1))
                    srow = rowp.tile([1, NT], F32, tag="row")
                    nc.scalar.activation(srow[:], prow[:], AF.Sqrt,
                                         bias=eps_sb[:], scale=1.0 / D)
                    rrow = rowp.tile([1, NT], F32, tag="row")
                    nc.vector.reciprocal(rrow[:], srow[:])
                    bc = bcp.tile([P, NT], F32, tag="bc")
                    nc.gpsimd.partition_broadcast(bc[:], rrow[:])
                    dst = actsr.tile([P, CC, NT], dt_out, tag="bigr")
                    for cc in range(CC):
                        nc.vector.tensor_mul(dst[:, cc, :], src_sb[:, cc, :], bc[:])
                    return dst

                xn = rmsnorm(xT, WDT)

                # ------------- q/k/v projections -> AllToAll bounce -------------
                # shard j (head pair j), row-major per partition p:
                #   [q^T 512 | k^T 512 | v 4x(2x65)]  (1544 bf16 per p-row)
                # one contiguous DMA per (batch, rank) on the attention side.
                # slot 64 of each v 65-group becomes 1.0 via on-chip memset.
                VW = 2 * 65          # 130 bf16 per (p, c)
                QKW = 2 * NT                   # 1024 bf16 per p-row (q|k)
                VVW = 4 * VW                   # 520 bf16 per p-row (v)
                qk_in = dram.tile([N_CORES, P * QKW], BF16)
                qk_out = dram.tile([N_CORES, P * QKW], BF16)
                v_in = dram.tile([N_CORES, P * VVW], BF16)
                v_out = dram.tile([N_CORES, P * VVW], BF16)

                def wmm(pb, wt_sl, xn_sl_pairs, j, njj):
                    """one contraction step: DR pair in fp8 mode, single bf16."""
                    if fp8_qkv:
                        nc.tensor.matmul(pb, wt_sl, xn_sl_pairs,
                                         start=(j == 0), stop=(j == njj - 1),
                                         perf_mode=DRmode)
                    else:
                        nc.tensor.matmul(pb, wt_sl, xn_sl_pairs,
                                         start=(j == 0), stop=(j == njj - 1))

                def proj_qk(w, region):
                    # whole weight tensor in one DMA (8 KB per partition)
                    wt = wstr.tile([P, CC, CC, P], WDT, tag="w81d", bufs=1)
                    nc.sync.dma_start(
                        wt[:], w.rearrange("f p c d -> p f (c d)"))
                    qcol = cpp.tile([P, CC, NT], BF16, tag="qcol", bufs=1)
                    for dt in range(CC):
                        pb = psb.tile([P, NT], F32, tag="pbig")
                        if fp8_qkv:
                            for j in range(CC // 2):
                                wmm(pb[:], wt[:, dt, 2 * j:2 * j + 2, :],
                                    xn[:, 2 * j:2 * j + 2, :], j, CC // 2)
                        else:
                            for cc in range(CC):
                                wmm(pb[:], wt[:, dt, cc, :],
                                    xn[:, cc, :], cc, CC)
                        nc.vector.tensor_copy(qcol[:, dt, :], pb[:])
                    # all 8 head-pair shards in one DMA
                    nc.sync.dma_start(
                        qk_in[:, :].rearrange("r (p z) -> p r z", z=QKW)
                        [:, :, region * NT:(region + 1) * NT], qcol[:])

                if "qkv" not in ABLATE:
                    proj_qk(wq, 0)
                    proj_qk(wk, 1)
                if with_collective:
                    nc.gpsimd.collective_compute(
                        "AllToAll", mybir.AluOpType.bypass, replica_groups=ALL8,
                        ins=[qk_in[:].opt()], outs=[qk_out[:].opt()])
                else:
                    nc.sync.dma_start(qk_out[:], qk_in[:])

                # v token-major: v[t, d] = sum_c xn[c, t] wv[c, d]
                for dt in range(2 if "qkv" not in ABLATE else 0):
                    wt = wstr2.tile([P, CC, NT], WDT, tag="w82")
                    nc.sync.dma_start(
                        wt[:], wv[:, dt * NT:(dt + 1) * NT]
                        .rearrange("(cc p) d -> p cc d", p=P))
                    for tt in range(4):
                        pb = psb.tile([P, NT], F32, tag="pbig")
                        if fp8_qkv:
                            for j in range(CC // 2):
                                wmm(pb[:],
                                    xn[:, 2 * j:2 * j + 2, tt * P:(tt + 1) * P],
                                    wt[:, 2 * j:2 * j + 2, :], j, CC // 2)
                        else:
                            for cc in range(CC):
                                wmm(pb[:], xn[:, cc, tt * P:(tt + 1) * P],
                                    wt[:, cc, :], cc, CC)
                        cp = cpp.tile([P, NT], BF16, tag="cpb")
                        nc.vector.tensor_copy(cp[:], pb[:])
                        for hh in range(2):
                            vdst = (v_in[4 * dt:4 * dt + 4]
                                    .rearrange("u (p z) -> p u z", z=VVW)
                                    [:, :, tt * VW:(tt + 1) * VW]
                                    .rearrange("p u (h e) -> p u h e", e=65)
                                    [:, :, hh, 0:DH])
                            nc.sync.dma_start(
                                vdst, cp[:].rearrange("p (u h d) -> p h u d",
                                                      h=2, d=DH)[:, hh])

                if with_collective:
                    nc.gpsimd.collective_compute(
                        "AllToAll", mybir.AluOpType.bypass, replica_groups=ALL8,
                        ins=[v_in[:].opt()], outs=[v_out[:].opt()])
                else:
                    nc.sync.dma_start(v_out[:], v_in[:])

                # loads that are only needed later: their DMA overlaps the
                # collective / the attention window instead of the qkv window
                mask_sb = constp.tile([P, 4, NT], BF16)
                nc.sync.dma_start(mask_sb[:], dmask.rearrange("g p q -> p g q"))
                wo_sb = constp.tile([P, CC, D], BF16)
                nc.sync.dma_start(wo_sb[:], wo[:])

                # ------------- attention (my 2 heads, all tokens) -------------
                a2a2_in = dram.tile([N_CORES, PNT], BF16)
                a2a2_out = dram.tile([N_CORES, PNT], BF16)
                yTm = actsr.tile([P, CC, NT], BF16, tag="bigr")
                exp_scale = 0.125 / (WS * WS) if fp8_qkv else 0.125
                for b in range(B):
                    # one DMA per batch for all 4 ranks' q|k planes and v
                    kq_t = kvp.tile([P, 4, QKW], BF16, tag="kq", bufs=2)
                    nc.sync.dma_start(
                        kq_t[:], qk_out[4 * b:4 * b + 4]
                        .rearrange("r (p z) -> p r z", z=QKW))
                    vv_t = kvp.tile([P, 4, VVW], BF16, tag="vv", bufs=2)
                    nc.sync.dma_start(
                        vv_t[:], v_out[4 * b:4 * b + 4]
                        .rearrange("r (p z) -> p r z", z=VVW))
                    nc.vector.memset(
                        vv_t[:].rearrange("p r (c h e) -> p (r c h) e",
                                          h=2, e=65)
                        [:, :, DH:DH + 1], 1.0)

                    for qb in range(4):
                        if "attn" in ABLATE:
                            continue
                        py0 = psy.tile([DH + 1, NT], F32, tag="py")
                        py1 = psy.tile([DH + 1, NT], F32, tag="py")
                        pys = (py0, py1)
                        nch = 4 * qb + 4
                        for g in range(nch):
                            # both heads' scores side by side in one 2-bank tile:
                            # the K=64 matmuls run in disjoint PE row groups and
                            # one ACT exp covers both.  Diagonal chunks only
                            # compute the live query suffix [qo:].
                            qo = P * (g - 4 * qb) if g >= 4 * qb else 0
                            rr = g // 4
                            ko = NT + (g % 4) * P
                            ps2 = pss.tile([P, 2, NT], F32, tag="ps")
                            nc.tensor.matmul(ps2[:, 0, qo:],
                                             kq_t[0:64, rr, ko:ko + P],
                                             kq_t[0:64, qb, qo:NT],
                                             start=True, stop=True)
                            nc.tensor.matmul(ps2[:, 1, qo:],
                                             kq_t[64:128, rr, ko:ko + P],
                                             kq_t[64:128, qb, qo:NT],
                                             start=True, stop=True)
                            es2 = expp.tile([P, 2, NT], BF16, tag="es")
                            nc.scalar.activation(es2[:, :, qo:], ps2[:, :, qo:],
                                                 AF.Exp, scale=exp_scale)
                            if g >= 4 * qb:
                                d = g - 4 * qb
                                nc.vector.tensor_mul(
                                    es2[:], es2[:],
                                    mask_sb[:, d:d + 1, :].to_broadcast([P, 2, NT]))
                            vo = (g % 4) * VW
                            nc.tensor.matmul(py0[:], vv_t[:, rr, vo:vo + 65],
                                             es2[:, 0, :],
                                             start=(g == 0), stop=(g == nch - 1))
                            nc.tensor.matmul(py1[:], vv_t[:, rr, vo + 65:vo + VW],
                                             es2[:, 1, :],
                                             start=(g == 0), stop=(g == nch - 1))
                        for hh in range(2):
                            lo = 64 * hh
                            pc = smallp.tile([DH + 1, NT], F32, tag="pyc")
                            nc.vector.tensor_copy(pc[:], pys[hh][:])
                            rrow = rowp.tile([1, NT], F32, tag="row")
                            nc.vector.reciprocal(rrow[:], pc[DH:DH + 1, :])
                            if fp8_qkv:
                                # v carries the WS weight scale; fold 1/WS into
                                # the per-token normalizer row
                                nc.vector.tensor_scalar_mul(rrow[:], rrow[:],
                                                            1.0 / WS)
                            bcy = smallp.tile([64, NT], F32, tag="abc")
                            nc.gpsimd.partition_broadcast(bcy[:], rrow[:])
                            nc.vector.tensor_mul(
                                yTm[lo:lo + 64, 4 * b + qb, :],
                                pc[0:DH, :], bcy[:])
                    if "attn" not in ABLATE:
                        nc.sync.dma_start(
                            a2a2_in[4 * b:4 * b + 4].rearrange(
                                "r (p t) -> p r t", t=NT),
                            yTm[:, 4 * b:4 * b + 4, :])

                # ------------- AllToAll y back to token shards -------------
                if "attn" in ABLATE:
                    nc.vector.memset(yTm[:], 0.0)
                    nc.sync.dma_start(
                        a2a2_in[:, :].rearrange("r (p t) -> p r t", t=NT), yTm[:])
                if with_collective:
                    nc.gpsimd.collective_compute(
                        "AllToAll", mybir.AluOpType.bypass, replica_groups=ALL8,
                        ins=[a2a2_in[:].opt()], outs=[a2a2_out[:].opt()])
                else:
                    nc.sync.dma_start(a2a2_out[:], a2a2_in[:])
                yTf = actsr.tile([P, CC, NT], BF16, tag="bigr")
                nc.sync.dma_start(
                    yTf[:], a2a2_out[:, :].rearrange("r (p t) -> p r t", t=NT))

                # ------------- o-proj + residual (in place into xT) -------------
                for ct in range(CC):
                    pb = psb.tile([P, NT], F32, tag="pbig")
                    for cc in range(CC):
                        nc.tensor.matmul(pb[:], wo_sb[:, cc, ct * P:(ct + 1) * P],
                                         yTf[:, cc, :],
                                         start=(cc == 0), stop=(cc == CC - 1))
                    nc.vector.tensor_add(xT[:, ct, :], pb[:], xT[:, ct, :])

                # ---------------- rmsnorm2 + fc1 + gelu ----------------
                xn2 = rmsnorm(xT, BF16)
                hT = hpool.tile([P, FT, NT], BF16)
                if "mlp" in ABLATE:
                    nc.vector.memset(hT[:, 0, :], 0.0)
                for fq in range(FT // 4 if "mlp" not in ABLATE else 0):
                    wt = wstr.tile([P, 4, CC, P], BF16, tag="w81q")
                    nc.sync.dma_start(
                        wt[:], w1[4 * fq:4 * fq + 4]
                        .rearrange("f p c d -> p f (c d)"))
                    for f4 in range(4):
                        ff = 4 * fq + f4
                        pb = psb.tile([P, NT], F32, tag="pbig")
                        for cc in range(CC):
                            nc.tensor.matmul(pb[:], wt[:, f4, cc, :], xn2[:, cc, :],
                                             start=(cc == 0), stop=(cc == CC - 1))
                        nc.scalar.activation(hT[:, ff, :], pb[:], AF.Gelu,
                                             bias=b1_sb[:, ff:ff + 1])

                # ---------------- fc2 + bias + residual ----------------
                # written in place into xT (dead after its last fc2 read),
                # then one merged store
                for ct in range(CC if "mlp" not in ABLATE else 0):
                    pb = psb.tile([P, NT], F32, tag="pbig")
                    wt = wstr2.tile([P, 2, FT // 2, P], BF16, tag="w16")
                    nc.sync.dma_start(
                        wt[:], w2[ct].rearrange("h p f c -> p h (f c)"))
                    for half in range(2):
                        for fo in range(FT // 2):
                            ffc = half * (FT // 2) + fo
                            nc.tensor.matmul(pb[:], wt[:, half, fo, :],
                                             hT[:, ffc, :],
                                             start=(ffc == 0), stop=(ffc == FT - 1))
                    nc.vector.tensor_add(xT[:, ct, :], pb[:], xT[:, ct, :])
                    nc.vector.tensor_scalar_add(xT[:, ct, :], xT[:, ct, :],
                                                b2_sb[:, ct:ct + 1])
                nc.sync.dma_start(
                    outT.rearrange("(cc p) t -> p cc t", p=P), xT[:])

    nc.compile()
    return nc


_CACHE = {}


def _get_compiled(with_collective=True, repeat=1, fp8_qkv=FP8_QKV):
    """Build the Bass program and a jitted 8-core PJRT executable once."""
    key = ("fn", with_collective, repeat, fp8_qkv)
    if key in _CACHE:
        return _CACHE[key]

    import jax
    from jax.sharding import Mesh, PartitionSpec
    from jax.experimental.shard_map import shard_map
    from concourse.bass2jax import (_bass_exec_p, install_neuronx_cc_hook,
                                    partition_id_tensor)

    nc = _build_nc(with_collective, repeat, fp8_qkv)
    install_neuronx_cc_hook()

    partition_name = (nc.partition_id_tensor.name
                      if nc.partition_id_tensor else None)
    in_names, out_names, out_avals = [], [], []
    for alloc in nc.m.functions[0].allocations:
        if not isinstance(alloc, mybir.MemoryLocationSet):
            continue
        name = alloc.memorylocations[0].name
        if alloc.kind == "ExternalInput":
            if name != partition_name:
                in_names.append(name)
        elif alloc.kind == "ExternalOutput":
            out_names.append(name)
            out_avals.append(jax.core.ShapedArray(
                tuple(alloc.tensor_shape), mybir.dt.np(alloc.dtype)))
    n_params = len(in_names)
    all_names = list(in_names) + list(out_names)
    if partition_name is not None:
        all_names.append(partition_name)

    def _body(*args):
        operands = list(args)
        if partition_name is not None:
            operands.append(partition_id_tensor())
        outs = _bass_exec_p.bind(
            *operands,
            out_avals=tuple(out_avals),
            in_names=tuple(all_names),
            out_names=tuple(out_names),
            lowering_input_output_aliases=(),
            sim_require_finite=True,
            sim_require_nnan=True,
            nc=nc,
        )
        return tuple(outs)

    devices = jax.devices()[:N_CORES]
    mesh = Mesh(np.asarray(devices), ("core",))
    sharded = jax.jit(shard_map(
        _body, mesh=mesh,
        in_specs=(PartitionSpec("core"),) * (n_params + len(out_names)),
        out_specs=(PartitionSpec("core"),) * len(out_names),
        check_rep=False))

    _CACHE[key] = (sharded, in_names, out_names, out_avals)
    return _CACHE[key]


def _host_inputs(x, Wq, Wk, Wv, Wo, W1, b1, W2, b2, g1, g2, fp8_qkv=FP8_QKV):
    """Per-core input dicts (all keys identically shaped across cores)."""
    bf = ml_dtypes.bfloat16
    f8 = ml_dtypes.float8_e4m3

    def tile8(w, dt):  # [D, out] -> [out_tiles, p, cc, 128] contiguous
        nt_ = w.shape[1] // P
        return np.ascontiguousarray(
            w.reshape(CC, P, nt_, P).transpose(2, 1, 0, 3)).astype(dt)

    if fp8_qkv:
        wq = tile8(WS * g1[:, None] * Wq, f8)
        wk = tile8(WS * g1[:, None] * Wk, f8)
        wv = (WS * g1[:, None] * Wv).astype(f8)
    else:
        wq = tile8(g1[:, None] * Wq, bf)
        wk = tile8(g1[:, None] * Wk, bf)
        wv = (g1[:, None] * Wv).astype(bf)
    w1 = tile8(g2[:, None] * W1, bf)
    w2 = np.ascontiguousarray(
        W2.reshape(2, FT // 2, P, CC, P).transpose(3, 0, 2, 1, 4)).astype(bf)
    wo = np.ascontiguousarray(
        Wo.reshape(CC, P, D).transpose(1, 0, 2)).astype(bf)
    b1 = np.ascontiguousarray(b1.reshape(FT, P).T).astype(np.float32)
    b2 = np.ascontiguousarray(b2.reshape(CC, P).T).astype(np.float32)
    onesc = np.ones((P, 1), np.float32)
    k_idx = np.arange(P)[:, None]
    q_idx = np.arange(NT)[None, :]
    dmask = np.stack([(P * d + k_idx <= q_idx) for d in range(4)]).astype(bf)

    per_core = []
    for c in range(N_CORES):
        b, s = divmod(c, 4)
        xt = np.ascontiguousarray(
            x[b, s * NT:(s + 1) * NT, :].T).astype(np.float32)
        per_core.append(dict(
            xt=xt, wq=wq, wk=wk, wv=wv, wo=wo,
            w1=w1, w2=w2, b1=b1, b2=b2,
            dmask=dmask, onesc=onesc))
    return per_core


def _concat_inputs(per_core, in_names, out_avals):
    concat = [np.concatenate([np.asarray(per_core[c][n])
                              for c in range(N_CORES)], axis=0)
              for n in in_names]
    concat += [np.zeros((N_CORES * a.shape[0], *a.shape[1:]), a.dtype)
               for a in out_avals]
    return concat


def _run(per_core):
    sharded, in_names, out_names, out_avals = _get_compiled()
    outs = sharded(*_concat_inputs(per_core, in_names, out_avals))
    res = np.asarray(outs[out_names.index("outT")])
    return res.reshape(N_CORES, D, NT)


def kernel(**inputs):
    np_in = {k: np.asarray(v) for k, v in inputs.items()}
    per_core = _host_inputs(**np_in)
    res = _run(per_core)
    out = np.empty((B, T, D), np.float32)
    for c in range(N_CORES):
        b, s = divmod(c, 4)
        out[b, s * NT:(s + 1) * NT, :] = res[c].T
    return out


def _bench_handles(inputs, with_collective=True, repeat=1, fp8_qkv=FP8_QKV):
    """Compiled runner + device-resident args, for benchmarking."""
    per_core = _host_inputs(
        **{k: np.asarray(v) for k, v in inputs.items()}, fp8_qkv=fp8_qkv)
    sharded, in_names, out_names, out_avals = _get_compiled(
        with_collective, repeat, fp8_qkv)
    import jax
    dev_args = [jax.device_put(a)
                for a in _concat_inputs(per_core, in_names, out_avals)]
    return sharded, dev_args


# revision 31
# speedup vs baseline: 1.5437x; 1.3403x over previous
"""Trainium2 Bass kernel for a dense transformer block (nn_Block_3564822855835).

Reference computation (fp32):
    x  = x + attention(rmsnorm(x, g1), Wq, Wk, Wv, Wo)   # causal MHA, 16 heads
    out = x + gelu(rmsnorm(x, g2) @ W1 + b1) @ W2 + b2   # exact-erf gelu

Shapes: x [2, 2048, 1024], 16 heads x 64, d_ff 4096.

Distribution (8 NeuronCores, one SPMD program, DeepSpeed-Ulysses style):
  - Token-sharded outside attention: core c owns batch c//4, token block
    c%4 (512 tokens): rmsnorm + q/k/v projections + o-proj + MLP all run
    on the core's own 512 tokens with full (replicated) weights.
  - Head-sharded inside attention: an 8-core AllToAll redistributes
    (q^T, k^T, v) from token-shards to head-shards; each core then runs
    causal attention for its 2 heads over the full 4096 tokens, and a
    second AllToAll routes y^T back to token shards.
  - Activations are channel-major ("x^T": [channel, token]) on-chip so all
    matmul contractions land on the partition axis with zero on-chip
    transposes.  Partition-dim reductions (rmsnorm sum, softmax sum) use
    a ones-column fused into the matmuls; scale rows are partition-
    broadcast on GPSIMD.
  - Diagonal causal chunks only compute the live query suffix: the score
    matmuls and the exp stream skip the fully-masked query prefix (the
    mask multiply still zeroes it for the AV accumulation).
  - Optionally (FP8_QKV) the q/k/v projections run in fp8 e4m3 with
    DoubleRow perf mode (2 contraction rows per PE pass, measured 2.2x
    matmul throughput on HW); weights are host-scaled by WS=128 and the
    descale rides the exp scale (q,k) and the softmax-normalizer row (v).
    The MLP and o-proj stay bf16: e4m3's ~4% per-matmul noise there takes
    the output past the accuracy budget.
  - Matmul operands are bf16/fp8 (fp32 PSUM accumulation); the residual
    stream and all softmax/norm statistics stay fp32.
"""

import numpy as np
import ml_dtypes

import concourse.bass as bass
import concourse.mybir as mybir
import concourse.tile as tile
from concourse import bacc

F32 = mybir.dt.float32
F32R = mybir.dt.float32r
BF16 = mybir.dt.bfloat16
F8 = mybir.dt.float8e4
AF = mybir.ActivationFunctionType
DRmode = mybir.MatmulPerfMode.DoubleRow

B, T, D = 2, 2048, 1024
H, DH = 16, 64
FF = 4096
EPS = 1e-6
P = 128
N_CORES = 8
NT = 512            # tokens per core
CC = D // P         # 8 channel chunks
NCH = T // P        # 16 k-chunks per batch
FT = FF // P        # 32 ff tiles
PNT = P * NT        # elements in one [128, 512] plane

WS = 128.0          # host-side fp8 weight scale (fp8 qkv mode)
FP8_QKV = True      # final config: q/k/v projections in fp8-DoubleRow

ALL8 = [[0, 1, 2, 3, 4, 5, 6, 7]]


ABLATE = set()


def _build_nc(with_collective=True, repeat=1, fp8_qkv=FP8_QKV):
    nc = bacc.Bacc("TRN2", target_bir_lowering=False, debug=False,
                   num_devices=N_CORES)

    def inp(name, shape, dt=F32):
        return nc.dram_tensor(name, shape, dt, kind="ExternalInput").ap()

    WDT = F8 if fp8_qkv else BF16
    xt = inp("xt", [D, NT])                 # x slice, channel-major
    # weights are host-pre-tiled so every DMA reads fully contiguous blocks
    wq = inp("wq", [CC, P, CC, P], WDT)     # [dt, p, cc, dd], g1-folded (x WS)
    wk = inp("wk", [CC, P, CC, P], WDT)     # [dt, p, cc, dd], g1-folded (x WS)
    wv = inp("wv", [D, D], WDT)             # g1-folded (x WS)
    wo = inp("wo", [P, CC, D], BF16)        # host-pretiled [p, cc, c]
    w1 = inp("w1", [FT, P, CC, P], BF16)    # [ff, p, cc, dd], g2-folded
    w2 = inp("w2", [CC, 2, P, FT // 2, P], BF16)  # [ct, half, p, fo, c]
    b1 = inp("b1", [P, FT])
    b2 = inp("b2", [P, CC])
    dmask = inp("dmask", [4, P, NT], BF16)  # diagonal causal masks (global)
    onesc = inp("onesc", [P, 1])
    outT = nc.dram_tensor("outT", [D, NT], F32, kind="ExternalOutput").ap()

    with tile.TileContext(nc) as tc:
        with tc.tile_pool(name="const", bufs=1) as constp, \
             tc.tile_pool(name="actsf", bufs=1) as actsf, \
             tc.tile_pool(name="actsr", bufs=3) as actsr, \
             tc.tile_pool(name="hpool", bufs=1) as hpool, \
             tc.tile_pool(name="wstr", bufs=2) as wstr, \
             tc.tile_pool(name="wstr2", bufs=2) as wstr2, \
             tc.tile_pool(name="kvp", bufs=6) as kvp, \
             tc.tile_pool(name="expp", bufs=5) as expp, \
             tc.tile_pool(name="cpp", bufs=3) as cpp, \
             tc.tile_pool(name="smallp", bufs=2) as smallp, \
             tc.tile_pool(name="rowp", bufs=2) as rowp, \
             tc.tile_pool(name="bcp", bufs=1) as bcp, \
             tc.tile_pool(name="psb", bufs=2, space="PSUM") as psb, \
             tc.tile_pool(name="pss", bufs=2, space="PSUM") as pss, \
             tc.tile_pool(name="psy", bufs=2, space="PSUM") as psy, \
             tc.tile_pool(name="dram", bufs=1, space="DRAM") as dram:

            # ---------------- constants ----------------
            onesc_sb = constp.tile([P, 1], F32R)
            nc.sync.dma_start(onesc_sb[:], onesc[:].bitcast(F32R))
            b1_sb = constp.tile([P, FT], F32)
            nc.sync.dma_start(b1_sb[:], b1[:])
            b2_sb = constp.tile([P, CC], F32)
            nc.sync.dma_start(b2_sb[:], b2[:])
            eps_sb = constp.tile([1, 1], F32)
            nc.vector.memset(eps_sb[:], EPS)
            onesr_sb = constp.tile([1, P], F32)
            nc.sync.dma_start(onesr_sb[:], onesc.rearrange("p one -> one p"))

            # warm all 5 es-pool buffers: diagonal score chunks only write the
            # live query suffix, and the masked prefix must multiply against
            # finite stale data (0 x NaN would poison the AV accumulation)
            for _ in range(5):
                es2 = expp.tile([P, 2, NT], BF16, tag="es")
                nc.vector.memset(es2[:], 0.0)

            for _rep in range(repeat):
                xT = actsf.tile([P, CC, NT], F32, tag="bigf")
                nc.sync.dma_start(xT[:], xt.rearrange("(cc p) t -> p cc t", p=P))

                # ---------------- rmsnorm (channel-major) ----------------
                def rmsnorm(src_sb, dt_out):
                    """src [P, CC, NT] f32 -> normalized [P, CC, NT] dt_out."""
                    prow = psb.tile([1, NT], F32, tag="pbig")
                    for cc in range(CC):
                        sq = smallp.tile([P, NT], F32R, tag="sq")
                        nc.vector.tensor_mul(sq[:], src_sb[:, cc, :], src_sb[:, cc, :])
                        nc.tensor.matmul(prow[:], onesc_sb[:], sq[:],
                                         start=(cc == 0), stop=(cc == CC - 1))
                    srow = rowp.tile([1, NT], F32, tag="row")
                    nc.scalar.activation(srow[:], prow[:], AF.Sqrt,
                                         bias=eps_sb[:], scale=1.0 / D)
                    rrow = rowp.tile([1, NT], F32, tag="row")
                    nc.vector.reciprocal(rrow[:], srow[:])
                    bc = bcp.tile([P, NT], F32, tag="bc")
                    nc.gpsimd.partition_broadcast(bc[:], rrow[:])
                    dst = actsr.tile([P, CC, NT], dt_out, tag="bigr")
                    for cc in range(CC):
                        nc.vector.tensor_mul(dst[:, cc, :], src_sb[:, cc, :], bc[:])
                    return dst

                xn = rmsnorm(xT, WDT)

                # ------------- q/k/v projections -> AllToAll bounce -------------
                # shard j (head pair j), row-major per partition p:
                #   [q^T 512 | k^T 512 | v 4x(2x65)]  (1544 bf16 per p-row)
                # one contiguous DMA per (batch, rank) on the attention side.
                # slot 64 of each v 65-group becomes 1.0 via on-chip memset.
                VW = 2 * 65          # 130 bf16 per (p, c)
                QKW = 2 * NT                   # 1024 bf16 per p-row (q|k)
                VVW = 4 * VW                   # 520 bf16 per p-row (v)
                qk_in = dram.tile([N_CORES, P * QKW], BF16)
                qk_out = dram.tile([N_CORES, P * QKW], BF16)
                v_in = dram.tile([N_CORES, P * VVW], BF16)
                v_out = dram.tile([N_CORES, P * VVW], BF16)

                def wmm(pb, wt_sl, xn_sl_pairs, j, njj):
                    """one contraction step: DR pair in fp8 mode, single bf16."""
                    if fp8_qkv:
                        nc.tensor.matmul(pb, wt_sl, xn_sl_pairs,
                                         start=(j == 0), stop=(j == njj - 1),
                                         perf_mode=DRmode)
                    else:
                        nc.tensor.matmul(pb, wt_sl, xn_sl_pairs,
                                         start=(j == 0), stop=(j == njj - 1))

                def proj_qk(w, region):
                    # whole weight tensor in one DMA (8 KB per partition)
                    wt = wstr.tile([P, CC, CC, P], WDT, tag="w81d", bufs=1)
                    nc.sync.dma_start(
                        wt[:], w.rearrange("f p c d -> p f (c d)"))
                    qcol = cpp.tile([P, CC, NT], BF16, tag="qcol", bufs=1)
                    for dt in range(CC):
                        pb = psb.tile([P, NT], F32, tag="pbig")
                        if fp8_qkv:
                            for j in range(CC // 2):
                                wmm(pb[:], wt[:, dt, 2 * j:2 * j + 2, :],
                                    xn[:, 2 * j:2 * j + 2, :], j, CC // 2)
                        else:
                            for cc in range(CC):
                                wmm(pb[:], wt[:, dt, cc, :],
                                    xn[:, cc, :], cc, CC)
                        nc.vector.tensor_copy(qcol[:, dt, :], pb[:])
                    # all 8 head-pair shards in one DMA
                    nc.sync.dma_start(
                        qk_in[:, :].rearrange("r (p z) -> p r z", z=QKW)
                        [:, :, region * NT:(region + 1) * NT], qcol[:])

                if "qkv" not in ABLATE:
                    proj_qk(wq, 0)
                    proj_qk(wk, 1)
                if with_collective:
                    nc.gpsimd.collective_compute(
                        "AllToAll", mybir.AluOpType.bypass, replica_groups=ALL8,
                        ins=[qk_in[:].opt()], outs=[qk_out[:].opt()])
                else:
                    nc.sync.dma_start(qk_out[:], qk_in[:])

                # v token-major: v[t, d] = sum_c xn[c, t] wv[c, d]
                for dt in range(2 if "qkv" not in ABLATE else 0):
                    wt = wstr2.tile([P, CC, NT], WDT, tag="w82")
                    nc.sync.dma_start(
                        wt[:], wv[:, dt * NT:(dt + 1) * NT]
                        .rearrange("(cc p) d -> p cc d", p=P))
                    for tt in range(4):
                        pb = psb.tile([P, NT], F32, tag="pbig")
                        if fp8_qkv:
                            for j in range(CC // 2):
                                wmm(pb[:],
                                    xn[:, 2 * j:2 * j + 2, tt * P:(tt + 1) * P],
                                    wt[:, 2 * j:2 * j + 2, :], j, CC // 2)
                        else:
                            for cc in range(CC):
                                wmm(pb[:], xn[:, cc, tt * P:(tt + 1) * P],
                                    wt[:, cc, :], cc, CC)
                        cp = cpp.tile([P, NT], BF16, tag="cpb")
                        nc.vector.tensor_copy(cp[:], pb[:])
                        for hh in range(2):
                            vdst = (v_in[4 * dt:4 * dt + 4]
                                    .rearrange("u (p z) -> p u z", z=VVW)
                                    [:, :, tt * VW:(tt + 1) * VW]
                                    .rearrange("p u (h e) -> p u h e", e=65)
                                    [:, :, hh, 0:DH])
                            nc.sync.dma_start(
                                vdst, cp[:].rearrange("p (u h d) -> p h u d",
                                                      h=2, d=DH)[:, hh])

                if with_collective:
                    nc.gpsimd.collective_compute(
                        "AllToAll", mybir.AluOpType.bypass, replica_groups=ALL8,
                        ins=[v_in[:].opt()], outs=[v_out[:].opt()])
                else:
                    nc.sync.dma_start(v_out[:], v_in[:])

                # loads that are only needed later: their DMA overlaps the
                # collective / the attention window instead of the qkv window
                mask_sb = constp.tile([P, 4, NT], BF16)
                nc.sync.dma_start(mask_sb[:], dmask.rearrange("g p q -> p g q"))
                wo_sb = constp.tile([P, CC, D], BF16)
                nc.sync.dma_start(wo_sb[:], wo[:])

                # ------------- attention (my 2 heads, all tokens) -------------
                a2a2_in = dram.tile([N_CORES, PNT], BF16)
                a2a2_out = dram.tile([N_CORES, PNT], BF16)
                yTm = actsr.tile([P, CC, NT], BF16, tag="bigr")
                exp_scale = 0.125 / (WS * WS) if fp8_qkv else 0.125
                for b in range(B):
                    # one DMA per batch for all 4 ranks' q|k planes and v
                    kq_t = kvp.tile([P, 4, QKW], BF16, tag="kq", bufs=2)
                    nc.sync.dma_start(
                        kq_t[:], qk_out[4 * b:4 * b + 4]
                        .rearrange("r (p z) -> p r z", z=QKW))
                    vv_t = kvp.tile([P, 4, VVW], BF16, tag="vv", bufs=2)
                    nc.sync.dma_start(
                        vv_t[:], v_out[4 * b:4 * b + 4]
                        .rearrange("r (p z) -> p r z", z=VVW))
                    nc.vector.memset(
                        vv_t[:].rearrange("p r (c h e) -> p (r c h) e",
                                          h=2, e=65)
                        [:, :, DH:DH + 1], 1.0)

                    for qb in range(4):
                        if "attn" in ABLATE:
                            continue
                        py0 = psy.tile([DH + 1, NT], F32, tag="py")
                        py1 = psy.tile([DH + 1, NT], F32, tag="py")
                        pys = (py0, py1)
                        nch = 4 * qb + 4
                        for g in range(nch):
                            # both heads' scores side by side in one 2-bank tile:
                            # the K=64 matmuls run in disjoint PE row groups and
                            # one ACT exp covers both.  Diagonal chunks only
                            # compute the live query suffix [qo:].
                            qo = P * (g - 4 * qb) if g >= 4 * qb else 0
                            rr = g // 4
                            ko = NT + (g % 4) * P
                            ps2 = pss.tile([P, 2, NT], F32, tag="ps")
                            nc.tensor.matmul(ps2[:, 0, qo:],
                                             kq_t[0:64, rr, ko:ko + P],
                                             kq_t[0:64, qb, qo:NT],
                                             start=True, stop=True)
                            nc.tensor.matmul(ps2[:, 1, qo:],
                                             kq_t[64:128, rr, ko:ko + P],
                                             kq_t[64:128, qb, qo:NT],
                                             start=True, stop=True)
                            es2 = expp.tile([P, 2, NT], BF16, tag="es")
                            nc.scalar.activation(es2[:, :, qo:], ps2[:, :, qo:],
                                                 AF.Exp, scale=exp_scale)
                            if g >= 4 * qb:
                                d = g - 4 * qb
                                nc.vector.tensor_mul(
                                    es2[:], es2[:],
                                    mask_sb[:, d:d + 1, :].to_broadcast([P, 2, NT]))
                            vo = (g % 4) * VW
                            nc.tensor.matmul(py0[:], vv_t[:, rr, vo:vo + 65],
                                             es2[:, 0, :],
                                             start=(g == 0), stop=(g == nch - 1))
                            nc.tensor.matmul(py1[:], vv_t[:, rr, vo + 65:vo + VW],
                                             es2[:, 1, :],
                                             start=(g == 0), stop=(g == nch - 1))
                        for hh in range(2):
                            lo = 64 * hh
                            pc = smallp.tile([DH + 1, NT], F32, tag="pyc")
                            nc.vector.tensor_copy(pc[:], pys[hh][:])
                            rrow = rowp.tile([1, NT], F32, tag="row")
                            nc.vector.reciprocal(rrow[:], pc[DH:DH + 1, :])
                            if fp8_qkv:
                                # v carries the WS weight scale; fold 1/WS into
                                # the per-token normalizer row
                                nc.vector.tensor_scalar_mul(rrow[:], rrow[:],
                                                            1.0 / WS)
                            # partition-broadcast via K=1 matmul into psb
                            # (idle during attention): keeps the py-drain off
                            # the slow GPSIMD queue so psy buffers recycle fast
                            bps = psb.tile([P, NT], F32, tag="pbig")
                            nc.tensor.matmul(bps[0:64, :], onesr_sb[:, 0:64],
                                             rrow[:], start=True, stop=True)
                            nc.vector.tensor_mul(
                                yTm[lo:lo + 64, 4 * b + qb, :],
                                pc[0:DH, :], bps[0:64, :])
                    if "attn" not in ABLATE:
                        nc.sync.dma_start(
                            a2a2_in[4 * b:4 * b + 4].rearrange(
                                "r (p t) -> p r t", t=NT),
                            yTm[:, 4 * b:4 * b + 4, :])

                # ------------- AllToAll y back to token shards -------------
                if "attn" in ABLATE:
                    nc.vector.memset(yTm[:], 0.0)
                    nc.sync.dma_start(
                        a2a2_in[:, :].rearrange("r (p t) -> p r t", t=NT), yTm[:])
                if with_collective:
                    nc.gpsimd.collective_compute(
                        "AllToAll", mybir.AluOpType.bypass, replica_groups=ALL8,
                        ins=[a2a2_in[:].opt()], outs=[a2a2_out[:].opt()])
                else:
                    nc.sync.dma_start(a2a2_out[:], a2a2_in[:])
                yTf = actsr.tile([P, CC, NT], BF16, tag="bigr")
                nc.sync.dma_start(
                    yTf[:], a2a2_out[:, :].rearrange("r (p t) -> p r t", t=NT))

                # ------------- o-proj + residual (in place into xT) -------------
                for ct in range(CC):
                    pb = psb.tile([P, NT], F32, tag="pbig")
                    for cc in range(CC):
                        nc.tensor.matmul(pb[:], wo_sb[:, cc, ct * P:(ct + 1) * P],
                                         yTf[:, cc, :],
                                         start=(cc == 0), stop=(cc == CC - 1))
                    nc.vector.tensor_add(xT[:, ct, :], pb[:], xT[:, ct, :])

                # ---------------- rmsnorm2 + fc1 + gelu ----------------
                xn2 = rmsnorm(xT, BF16)
                hT = hpool.tile([P, FT, NT], BF16)
                if "mlp" in ABLATE:
                    nc.vector.memset(hT[:, 0, :], 0.0)
                for fq in range(FT // 4 if "mlp" not in ABLATE else 0):
                    wt = wstr.tile([P, 4, CC, P], BF16, tag="w81q")
                    nc.sync.dma_start(
                        wt[:], w1[4 * fq:4 * fq + 4]
                        .rearrange("f p c d -> p f (c d)"))
                    for f4 in range(4):
                        ff = 4 * fq + f4
                        pb = psb.tile([P, NT], F32, tag="pbig")
                        for cc in range(CC):
                            nc.tensor.matmul(pb[:], wt[:, f4, cc, :], xn2[:, cc, :],
                                             start=(cc == 0), stop=(cc == CC - 1))
                        nc.scalar.activation(hT[:, ff, :], pb[:], AF.Gelu,
                                             bias=b1_sb[:, ff:ff + 1])

                # ---------------- fc2 + bias + residual ----------------
                # written in place into xT (dead after its last fc2 read),
                # then one merged store
                for ct in range(CC if "mlp" not in ABLATE else 0):
                    pb = psb.tile([P, NT], F32, tag="pbig")
                    wt = wstr2.tile([P, 2, FT // 2, P], BF16, tag="w16")
                    nc.sync.dma_start(
                        wt[:], w2[ct].rearrange("h p f c -> p h (f c)"))
                    for half in range(2):
                        for fo in range(FT // 2):
                            ffc = half * (FT // 2) + fo
                            nc.tensor.matmul(pb[:], wt[:, half, fo, :],
                                             hT[:, ffc, :],
                                             start=(ffc == 0), stop=(ffc == FT - 1))
                    nc.vector.tensor_add(xT[:, ct, :], pb[:], xT[:, ct, :])
                    nc.vector.tensor_scalar_add(xT[:, ct, :], xT[:, ct, :],
                                                b2_sb[:, ct:ct + 1])
                nc.sync.dma_start(
                    outT.rearrange("(cc p) t -> p cc t", p=P), xT[:])

    nc.compile()
    return nc


_CACHE = {}


def _get_compiled(with_collective=True, repeat=1, fp8_qkv=FP8_QKV):
    """Build the Bass program and a jitted 8-core PJRT executable once."""
    key = ("fn", with_collective, repeat, fp8_qkv)
    if key in _CACHE:
        return _CACHE[key]

    import jax
    from jax.sharding import Mesh, PartitionSpec
    from jax.experimental.shard_map import shard_map
    from concourse.bass2jax import (_bass_exec_p, install_neuronx_cc_hook,
                                    partition_id_tensor)

    nc = _build_nc(with_collective, repeat, fp8_qkv)
    install_neuronx_cc_hook()

    partition_name = (nc.partition_id_tensor.name
                      if nc.partition_id_tensor else None)
    in_names, out_names, out_avals = [], [], []
    for alloc in nc.m.functions[0].allocations:
        if not isinstance(alloc, mybir.MemoryLocationSet):
            continue
        name = alloc.memorylocations[0].name
        if alloc.kind == "ExternalInput":
            if name != partition_name:
                in_names.append(name)
        elif alloc.kind == "ExternalOutput":
            out_names.append(name)
            out_avals.append(jax.core.ShapedArray(
                tuple(alloc.tensor_shape), mybir.dt.np(alloc.dtype)))
    n_params = len(in_names)
    all_names = list(in_names) + list(out_names)
    if partition_name is not None:
        all_names.append(partition_name)

    def _body(*args):
        operands = list(args)
        if partition_name is not None:
            operands.append(partition_id_tensor())
        outs = _bass_exec_p.bind(
            *operands,
            out_avals=tuple(out_avals),
            in_names=tuple(all_names),
            out_names=tuple(out_names),
            lowering_input_output_aliases=(),
            sim_require_finite=True,
            sim_require_nnan=True,
            nc=nc,
        )
        return tuple(outs)

    devices = jax.devices()[:N_CORES]
    mesh = Mesh(np.asarray(devices), ("core",))
    sharded = jax.jit(shard_map(
        _body, mesh=mesh,
        in_specs=(PartitionSpec("core"),) * (n_params + len(out_names)),
        out_specs=(PartitionSpec("core"),) * len(out_names),
        check_rep=False))

    _CACHE[key] = (sharded, in_names, out_names, out_avals)
    return _CACHE[key]


def _host_inputs(x, Wq, Wk, Wv, Wo, W1, b1, W2, b2, g1, g2, fp8_qkv=FP8_QKV):
    """Per-core input dicts (all keys identically shaped across cores)."""
    bf = ml_dtypes.bfloat16
    f8 = ml_dtypes.float8_e4m3

    def tile8(w, dt):  # [D, out] -> [out_tiles, p, cc, 128] contiguous
        nt_ = w.shape[1] // P
        return np.ascontiguousarray(
            w.reshape(CC, P, nt_, P).transpose(2, 1, 0, 3)).astype(dt)

    if fp8_qkv:
        wq = tile8(WS * g1[:, None] * Wq, f8)
        wk = tile8(WS * g1[:, None] * Wk, f8)
        wv = (WS * g1[:, None] * Wv).astype(f8)
    else:
        wq = tile8(g1[:, None] * Wq, bf)
        wk = tile8(g1[:, None] * Wk, bf)
        wv = (g1[:, None] * Wv).astype(bf)
    w1 = tile8(g2[:, None] * W1, bf)
    w2 = np.ascontiguousarray(
        W2.reshape(2, FT // 2, P, CC, P).transpose(3, 0, 2, 1, 4)).astype(bf)
    wo = np.ascontiguousarray(
        Wo.reshape(CC, P, D).transpose(1, 0, 2)).astype(bf)
    b1 = np.ascontiguousarray(b1.reshape(FT, P).T).astype(np.float32)
    b2 = np.ascontiguousarray(b2.reshape(CC, P).T).astype(np.float32)
    onesc = np.ones((P, 1), np.float32)
    k_idx = np.arange(P)[:, None]
    q_idx = np.arange(NT)[None, :]
    dmask = np.stack([(P * d + k_idx <= q_idx) for d in range(4)]).astype(bf)

    per_core = []
    for c in range(N_CORES):
        b, s = divmod(c, 4)
        xt = np.ascontiguousarray(
            x[b, s * NT:(s + 1) * NT, :].T).astype(np.float32)
        per_core.append(dict(
            xt=xt, wq=wq, wk=wk, wv=wv, wo=wo,
            w1=w1, w2=w2, b1=b1, b2=b2,
            dmask=dmask, onesc=onesc))
    return per_core


def _concat_inputs(per_core, in_names, out_avals):
    concat = [np.concatenate([np.asarray(per_core[c][n])
                              for c in range(N_CORES)], axis=0)
              for n in in_names]
    concat += [np.zeros((N_CORES * a.shape[0], *a.shape[1:]), a.dtype)
               for a in out_avals]
    return concat


def _run(per_core):
    sharded, in_names, out_names, out_avals = _get_compiled()
    outs = sharded(*_concat_inputs(per_core, in_names, out_avals))
    res = np.asarray(outs[out_names.index("outT")])
    return res.reshape(N_CORES, D, NT)


def kernel(**inputs):
    np_in = {k: np.asarray(v) for k, v in inputs.items()}
    per_core = _host_inputs(**np_in)
    res = _run(per_core)
    out = np.empty((B, T, D), np.float32)
    for c in range(N_CORES):
        b, s = divmod(c, 4)
        out[b, s * NT:(s + 1) * NT, :] = res[c].T
    return out


def _bench_handles(inputs, with_collective=True, repeat=1, fp8_qkv=FP8_QKV):
    """Compiled runner + device-resident args, for benchmarking."""
    per_core = _host_inputs(
        **{k: np.asarray(v) for k, v in inputs.items()}, fp8_qkv=fp8_qkv)
    sharded, in_names, out_names, out_avals = _get_compiled(
        with_collective, repeat, fp8_qkv)
    import jax
    dev_args = [jax.device_put(a)
                for a in _concat_inputs(per_core, in_names, out_avals)]
    return sharded, dev_args
